# revision 13
# baseline (speedup 1.0000x reference)
"""GQA (B=1, S=2048, D=4096, H=32, G=8) on 8 TRN2 NeuronCores.

Sharding: tensor-parallel over heads — core c owns query heads 4c..4c+3 and
KV group c. Per core: bf16 qT/kT/vT projections from full x (transposed
layouts), RoPE, causal attention with transposed probs (exp without
max-subtraction — scores are bounded; denominator via a ones-column in V'),
normalization folded into a per-partition scalar multiply, PE-transpose of
ctx, per-chunk bf16 AllGather of ctxT, then a column-sharded Wo matmul with
Wo stationary / ctxT moving producing a transposed output tile. Host slices
weights / transposes x / assembles output columns.

Self-contained: no sibling imports; hardcoded shapes.
"""
import contextlib
import ctypes
import os
import sys
import types

import ml_dtypes
import numpy as np

os.environ.setdefault("MYCRO_LOCAL_CACHE", "1")

for _p in ("/opt/trn_rl_repo", "/root/.axon_site/_ro/trn_rl_repo"):
    if _p not in sys.path and os.path.isdir(_p):
        sys.path.append(_p)

import concourse.bass as bass
import concourse.tile as tile
from concourse import mybir
from concourse.bass_utils import run_bass_kernel_spmd
from concourse.masks import make_identity

# ---------------------------------------------------------------- profiling shim
_SO_PATH = "/opt/axon/libaxon_pjrt.so"
_hook_holder = [None]


def _ntff_profile_via_ctypes(so_path):
    try:
        lib = ctypes.CDLL(so_path)
    except OSError:
        return None
    if not hasattr(lib, "axon_start_nrt_profile"):
        return None
    lib.axon_start_nrt_profile.argtypes = [
        ctypes.POINTER(ctypes.c_int64),
        ctypes.c_size_t,
    ]
    lib.axon_start_nrt_profile.restype = ctypes.c_int64
    lib.axon_stop_nrt_profile.argtypes = [ctypes.c_char_p]
    lib.axon_stop_nrt_profile.restype = ctypes.c_int64

    @contextlib.contextmanager
    def _hook(output_dir, device_ids):
        import jax

        jax.devices()
        if device_ids:
            ids = (ctypes.c_int64 * len(device_ids))(*device_ids)
            rc = lib.axon_start_nrt_profile(ids, len(device_ids))
        else:
            rc = lib.axon_start_nrt_profile(None, 0)
        if rc != 0:
            raise RuntimeError(f"axon_start_nrt_profile rc={rc}")
        try:
            yield
        finally:
            n = lib.axon_stop_nrt_profile(str(output_dir).encode())
            if n <= 0:
                print(f"WARNING: ntff capture wrote {n} files", file=sys.stderr)

    return _hook


def _install_prof_shim():
    if "antenv.axon_hooks" not in sys.modules:
        mod = types.ModuleType("antenv.axon_hooks")
        mod.set_axon_ntff_profile_hook = lambda h: _hook_holder.__setitem__(0, h)
        mod.get_axon_ntff_profile_hook = lambda: _hook_holder[0]
        sys.modules["antenv.axon_hooks"] = mod
    _hook_holder[0] = _ntff_profile_via_ctypes(_SO_PATH)
    import concourse.bass_utils as bu

    bu.upload_artifacts = lambda tmpdir: tmpdir


_install_prof_shim()

# ------------------------------------------------------------- wait-split pass
def _split_multi_waits(nc, maxw=1):
    """walrus in this container allows only one sync-wait per instruction;
    split extras onto nops inserted before the offender (same engine/block)."""

    def _remove_by_name(name):
        for f in nc.m.functions:
            for bb in f.blocks:
                for i, inst in enumerate(bb.instructions):
                    if inst.name == name:
                        lst = bb.instructions
                        del lst[i]
                        bb.instructions = lst
                        return inst
        raise KeyError(name)

    offenders = []
    for f in nc.m.functions:
        for bb in f.blocks:
            for inst in bb.instructions:
                si = inst.sync_info
                if si and si.on_wait and len(si.on_wait) > maxw:
                    offenders.append(inst.name)
    for name in offenders:
        target = None
        for f in nc.m.functions:
            for bb in f.blocks:
                for idx, inst in enumerate(bb.instructions):
                    if inst.name == name:
                        target = (bb, inst)
                        break
                if target:
                    break
            if target:
                break
        bb, inst = target
        waits = list(inst.sync_info.on_wait)
        updates = list(inst.sync_info.on_update or [])
        chunks = [waits[i:i + maxw] for i in range(0, len(waits), maxw)]
        nops = []
        for ch in chunks[:-1]:
            bnop = nc.engines[inst.engine].nop(nofuse=True, hint="waitsplit")
            nop_inst = _remove_by_name(bnop.ins.name)
            nop_inst.sync_info = mybir.SyncInfo(on_wait=ch, on_update=[])
            nops.append(nop_inst)
        inst.sync_info = mybir.SyncInfo(on_wait=chunks[-1], on_update=updates)
        lst = bb.instructions
        idx = next(i for i, x in enumerate(lst) if x.name == name)
        lst[idx:idx] = nops
        bb.instructions = lst
    return len(offenders)


# ------------------------------------------------------------------- constants
B, S, D = 1, 2048, 4096
H, G = 32, 8
HD = D // H            # 128
NC = 8                 # cores
HPC = H // NC          # heads per core = 4
OC = D // NC           # out columns per core = 512
P = 128
KT = D // P            # 32 contraction tiles
SCH = 512              # sequence chunk width
NSC = S // SCH         # 4
NQ = SCH // P          # 4 query subtiles per chunk
NKB = S // P           # 16 key tiles
SCALE = float(1.0 / np.sqrt(np.float32(HD)))

f32 = mybir.dt.float32
bf16 = mybir.dt.bfloat16

Copy = mybir.ActivationFunctionType.Copy
Exp = mybir.ActivationFunctionType.Exp


def _build_program():
    nc = bass.Bass()
    xT = nc.declare_dram_parameter("xT", [P, KT, S], bf16, isOutput=False)
    wq = nc.declare_dram_parameter("wq", [P, KT, OC], bf16, isOutput=False)
    wk = nc.declare_dram_parameter("wk", [P, KT, HD], bf16, isOutput=False)
    wv = nc.declare_dram_parameter("wv", [P, KT, HD], bf16, isOutput=False)
    wo = nc.declare_dram_parameter("wo", [P, KT, OC], bf16, isOutput=False)
    cosT = nc.declare_dram_parameter("cosT", [HD, S], f32, isOutput=False)
    sinT = nc.declare_dram_parameter("sinT", [HD, S], f32, isOutput=False)
    tri = nc.declare_dram_parameter("tri", [P, P], bf16, isOutput=False)
    outT = nc.declare_dram_parameter("outT", [OC, S], f32, isOutput=True)

    cc_ins = [nc.dram_tensor(f"cc_in{k}", [HPC * HD, SCH], bf16)
              for k in range(NSC)]
    cc_outs = [nc.dram_tensor(f"cc_out{k}", [D, SCH], bf16, addr_space="Shared")
               for k in range(NSC)]

    with tile.TileContext(nc) as tc:
        with (
            tc.tile_pool(name="singles", bufs=1) as singles,
            tc.tile_pool(name="stream", bufs=12) as stream,
            tc.tile_pool(name="qts", bufs=6) as qtsp,
            tc.tile_pool(name="pt", bufs=18) as ptp,
            tc.tile_pool(name="work", bufs=6) as work,
            tc.tile_pool(name="evict", bufs=4) as evictp,
            tc.tile_pool(name="ps", bufs=1, space="PSUM") as psp,
        ):
            # ---- startup DMAs, ordered so the first K-proj matmul can start
            # as soon as wk + the first x tile land.
            trim = singles.tile([P, P], bf16)
            nc.sync.dma_start(out=trim[:], in_=tri[:])
            wk_sb = singles.tile([P, KT, HD], bf16)
            for hh in range(2):
                nc.sync.dma_start(out=wk_sb[:, 16 * hh:16 * hh + 16, :],
                                  in_=wk[:, 16 * hh:16 * hh + 16, :])
            wv_sb = singles.tile([P, KT, HD], bf16)
            for hh in range(2):
                nc.sync.dma_start(out=wv_sb[:, 16 * hh:16 * hh + 16, :],
                                  in_=wv[:, 16 * hh:16 * hh + 16, :])

            xtg0 = []
            for g in range(KT // 4):
                t = stream.tile([P, 4, SCH], bf16, tag="stream")
                if g == 0:
                    nc.sync.dma_start(out=t[:, 0:2, :], in_=xT[:, 0:2, 0:SCH])
                    nc.sync.dma_start(out=t[:, 2:4, :], in_=xT[:, 2:4, 0:SCH])
                else:
                    nc.sync.dma_start(out=t[:],
                                      in_=xT[:, 4 * g:4 * g + 4, 0:SCH])
                xtg0.append(t)

            cos_sb = singles.tile([HD, S], f32)
            nc.sync.dma_start(out=cos_sb[:], in_=cosT[:])
            sin_sb = singles.tile([HD, S], f32)
            nc.sync.dma_start(out=sin_sb[:], in_=sinT[:])

            wq_sb = singles.tile([P, KT, OC], bf16)
            for hh in range(4):
                nc.sync.dma_start(out=wq_sb[:, 8 * hh:8 * hh + 8, :],
                                  in_=wq[:, 8 * hh:8 * hh + 8, :])
            wo_sb = singles.tile([P, KT, OC], bf16)
            for hh in range(2):
                nc.sync.dma_start(out=wo_sb[:, 16 * hh:16 * hh + 16, :],
                                  in_=wo[:, 16 * hh:16 * hh + 16, :])

            identb = singles.tile([P, P], bf16)
            make_identity(nc, identb[:])

            kT_all = singles.tile([HD, S], bf16)
            vp_all = singles.tile([P, NKB, HD + 1], bf16)
            nc.vector.memset(vp_all[:], 1.0)

            def rope_evict(ps_t, dst, dst0, tab0):
                """ps_t: PSUM [HD, SCH] pre-rope; writes dst[:, dst0:dst0+SCH]
                (bf16) using rope tables at absolute position tab0."""
                rot = work.tile([HD, SCH], f32, tag="rot", bufs=2)
                nc.scalar.activation(out=rot[0:64, :], in_=ps_t[64:128, :],
                                     func=Copy, scale=-1.0)
                nc.scalar.activation(out=rot[64:128, :], in_=ps_t[0:64, :],
                                     func=Copy)
                m1 = work.tile([HD, SCH], f32, tag="m1", bufs=2)
                nc.vector.tensor_mul(m1[:], ps_t[:], cos_sb[:, tab0:tab0 + SCH])
                nc.vector.tensor_mul(rot[:], rot[:], sin_sb[:, tab0:tab0 + SCH])
                nc.vector.tensor_add(dst[:, dst0:dst0 + SCH], m1[:], rot[:])

            # ================= phase 1: projections + attention, per s-chunk
            for sc in range(NSC):
                s0 = sc * SCH
                if sc == 0:
                    xtg = xtg0
                else:
                    xtg = []
                    for g in range(KT // 4):
                        t = stream.tile([P, 4, SCH], bf16, tag="stream")
                        nc.sync.dma_start(out=t[:],
                                          in_=xT[:, 4 * g:4 * g + 4,
                                               s0:s0 + SCH])
                        xtg.append(t)

                def xts(kt):
                    return xtg[kt // 4][:, kt % 4, :]

                # K projection -> RoPE -> kT_all
                ps_k = psp.tile([P, SCH], f32, tag="a", bufs=4)
                for kt in range(KT):
                    nc.tensor.matmul(ps_k[:], wk_sb[:, kt, :], xts(kt),
                                     start=(kt == 0), stop=(kt == KT - 1))
                rope_evict(ps_k, kT_all, s0, s0)

                # V projection (transposed) -> vp_all
                ps_v = psp.tile([P, SCH], f32, tag="a", bufs=4)
                for kt in range(KT):
                    nc.tensor.matmul(ps_v[:], wv_sb[:, kt, :], xts(kt),
                                     start=(kt == 0), stop=(kt == KT - 1))
                vc = work.tile([HD, SCH], bf16, tag="vc", bufs=2)
                nc.scalar.copy(vc[:], ps_v[:])
                for half in range(NQ):
                    kb = sc * NQ + half
                    ps_vt = psp.tile([P, P], bf16, tag="b", bufs=2)
                    nc.tensor.transpose(
                        ps_vt[:], vc[:, half * P:(half + 1) * P], identb[:]
                    )
                    nc.scalar.copy(vp_all[:, kb, 0:HD], ps_vt[:])

                # Q projections + RoPE (4 heads)
                qts = []
                for h in range(HPC):
                    ps_q = psp.tile([P, SCH], f32, tag="a", bufs=4)
                    for kt in range(KT):
                        nc.tensor.matmul(
                            ps_q[:], wq_sb[:, kt, h * P:(h + 1) * P], xts(kt),
                            start=(kt == 0), stop=(kt == KT - 1))
                    qt = qtsp.tile([HD, SCH], bf16, tag="qts")
                    rope_evict(ps_q, qt, 0, s0)
                    qts.append(qt)

                # attention for this chunk's queries: shared-history
                # scores first, then per-qh (diag scores, ctx) so each ctx
                # chain starts as soon as its last diagonal exp lands.
                for h in range(HPC):
                    pts = []

                    def score_tile(kb):
                        diag = kb - NQ * sc
                        c0 = max(0, diag) * P
                        ps_s = psp.tile([P, SCH], f32, tag="a", bufs=4)
                        nc.tensor.matmul(ps_s[:, c0:SCH],
                                         kT_all[:, kb * P:(kb + 1) * P],
                                         qts[h][:, c0:SCH],
                                         start=True, stop=True)
                        pt = ptp.tile([P, SCH], bf16, tag="pt")
                        nc.scalar.activation(out=pt[:, c0:SCH],
                                             in_=ps_s[:, c0:SCH],
                                             func=Exp, scale=SCALE)
                        if 0 <= diag:
                            nc.vector.tensor_mul(
                                pt[:, diag * P:(diag + 1) * P],
                                pt[:, diag * P:(diag + 1) * P], trim[:])
                        pts.append(pt)

                    for kb in range(NQ * sc):
                        score_tile(kb)

                    for qh in range(NQ):
                        iqc = NQ * sc + qh
                        score_tile(iqc)
                        ps_c = psp.tile([P, HD + 1], f32, tag="b", bufs=2)
                        for kb in range(iqc + 1):
                            nc.tensor.matmul(
                                ps_c[:], pts[kb][:, qh * P:(qh + 1) * P],
                                vp_all[:, kb, :],
                                start=(kb == 0), stop=(kb == iqc))
                        rden = work.tile([P, 1], f32, tag="rden", bufs=4)
                        nc.vector.reciprocal(rden[:], ps_c[:, HD:HD + 1])
                        ctxn = work.tile([P, HD], bf16, tag="ctxn", bufs=4)
                        nc.vector.tensor_scalar_mul(ctxn[:], ps_c[:, 0:HD],
                                                    rden[:])
                        ps_t = psp.tile([P, P], bf16, tag="b", bufs=2)
                        nc.tensor.transpose(ps_t[:], ctxn[:], identb[:])
                        ctxT_sb = evictp.tile([HD, P], bf16, tag="ctxT",
                                              bufs=4)
                        nc.scalar.copy(ctxT_sb[:], ps_t[:])
                        nc.sync.dma_start(
                            out=cc_ins[sc][h * HD:(h + 1) * HD,
                                           qh * P:(qh + 1) * P],
                            in_=ctxT_sb[:])

                nc.gpsimd.collective_compute(
                    "AllGather",
                    mybir.AluOpType.bypass,
                    replica_groups=[list(range(NC))],
                    ins=[cc_ins[sc][:]],
                    outs=[cc_outs[sc][:]],
                )

            # ================= phase 2: Wo (stationary) x ctxT (moving)
            cc3s = [cc_outs[k][:].rearrange("(t p) s -> p t s", p=P)
                    for k in range(NSC)]
            for spl in range(NSC):
                ccts = []
                for g in range(KT // 4):
                    t = stream.tile([P, 4, SCH], bf16, tag="stream")
                    nc.sync.dma_start(
                        out=t[:], in_=cc3s[spl][:, 4 * g:4 * g + 4, :])
                    ccts.append(t)
                for ob in range(OC // P):
                    ps_o = psp.tile([P, SCH], f32, tag="c", bufs=2)
                    for kt in range(KT):
                        nc.tensor.matmul(
                            ps_o[:], wo_sb[:, kt, ob * P:(ob + 1) * P],
                            ccts[kt // 4][:, kt % 4, :],
                            start=(kt == 0), stop=(kt == KT - 1))
                    out_sb = evictp.tile([P, SCH], f32, tag="osb", bufs=2)
                    nc.scalar.copy(out_sb[:], ps_o[:])
                    for hh in range(2):
                        nc.sync.dma_start(
                            out=outT[ob * P:(ob + 1) * P,
                                     spl * SCH + hh * 256:
                                     spl * SCH + (hh + 1) * 256],
                            in_=out_sb[:, hh * 256:(hh + 1) * 256])

    return nc


_PROGRAM_CACHE = {}


def _get_program():
    if "nc" not in _PROGRAM_CACHE:
        nc = _build_program()
        _split_multi_waits(nc, maxw=1)
        _PROGRAM_CACHE["nc"] = nc
    return _PROGRAM_CACHE["nc"]


def _rope_tables_T():
    inv_freq = (1.0 / (10000.0 ** (np.arange(0, HD, 2, dtype=np.float32) / HD))
                ).astype(np.float32)
    ang = np.arange(S, dtype=np.float32)[:, None] * inv_freq[None, :]
    ang = np.concatenate([ang, ang], axis=-1)  # [S, HD]
    return (np.ascontiguousarray(np.cos(ang).T.astype(np.float32)),
            np.ascontiguousarray(np.sin(ang).T.astype(np.float32)))


def _prep_in_maps(x, Wq, Wk, Wv, Wo):
    bf = ml_dtypes.bfloat16
    x2d = np.asarray(x, np.float32).reshape(S, D)
    xT_dev = np.ascontiguousarray(
        x2d.T.reshape(KT, P, S).transpose(1, 0, 2)).astype(bf)
    cosT, sinT = _rope_tables_T()
    tri_np = (np.arange(P)[:, None] <= np.arange(P)[None, :]).astype(bf)

    def wtiles(Wslice, width):
        return np.ascontiguousarray(
            np.asarray(Wslice, np.float32).reshape(KT, P, width)
            .transpose(1, 0, 2)).astype(bf)

    in_maps = []
    for c in range(NC):
        in_maps.append({
            "xT": xT_dev,
            "wq": wtiles(Wq[:, c * OC:(c + 1) * OC], OC),
            "wk": wtiles(Wk[:, c * HD:(c + 1) * HD], HD),
            "wv": wtiles(Wv[:, c * HD:(c + 1) * HD], HD),
            "wo": wtiles(Wo[:, c * OC:(c + 1) * OC], OC),
            "cosT": cosT,
            "sinT": sinT,
            "tri": tri_np,
        })
    return in_maps


def _run(inputs, trace=False):
    nc = _get_program()
    in_maps = _prep_in_maps(inputs["x"], inputs["Wq"], inputs["Wk"],
                            inputs["Wv"], inputs["Wo"])
    res = run_bass_kernel_spmd(nc, in_maps, core_ids=list(range(NC)),
                               trace=trace)
    out = np.empty((S, D), np.float32)
    for c in range(NC):
        out[:, c * OC:(c + 1) * OC] = res.results[c]["outT"].T
    return out.reshape(B, S, D), res


def kernel(**inputs):
    out, _ = _run(inputs, trace=False)
    return out
